# revision 12
# baseline (speedup 1.0000x reference)
"""CrossAttention kernel for 8 Trainium2 NeuronCores.

Sharding: 8 cores = 4 batches x 2 head-groups (8 heads each).
Core c handles batch b = c//2, head-group g = c%2.

Per-core device kernel (all matmuls fp32r):
  inputs : xT  = x[b].T                  [1024, 2048]
           wq/wk/wv = W*[:, g*512:+512]  [1024, 512]
           wo  = Wo[g*512:+512, :]       [512, 1024]
  output : outT = (O @ wo).T partial     [1024, 2048]
Host side: out[b] = (outT[2b] + outT[2b+1]).T + bo.

On-chip layouts (partition dim first):
  Q^T, K^T : [128 = 2 heads x 64, pair, 2048 tok]  (d-major)
  V        : [128 tok, tok_chunk, 512 hd]          (token-major)
  S^T tile : [128 kv-tok, 512 q-tok] per head; exp on ScalarE from PSUM
  O^T      : [128 = 2 heads x 64, chunk, 2048 tok]
Attention per head: S^T = K Q^T (2-head row-packed, K=64),
O^T = V^T P^T (2-head col-packed), denom = ones^T P^T (4-head col-packed),
normalization fused into the PSUM->SBUF copy of O^T.
"""

import sys

for _p in ("/opt/trn_rl_repo", "/opt/pypackages"):
    if _p not in sys.path:
        sys.path.append(_p)

import numpy as np

import concourse.bass as bass
import concourse.tile as tile
from concourse import bacc, mybir
from concourse.bass_utils import run_bass_kernel_spmd

N_CORES = 8
B, N, DIM = 4, 2048, 1024
HEADS, DH = 16, 64
SCALE = DH ** -0.5
HPC = HEADS // 2          # heads per core (8)
HDC = HPC * DH            # head dims per core (512)

P = 128                   # partitions
FD = 512                  # matmul free dim (fp32 PSUM bank)
CC = DIM // P             # contraction chunks for projections (8)
TT = N // FD              # token tiles (4)
TC = N // P               # token chunks (16)
HC = HDC // P             # head-dim chunks per core (4)
OC = DIM // P             # out-dim chunks (8)

F32 = mybir.dt.float32
F32R = mybir.dt.float32r
BF16 = mybir.dt.bfloat16


def build_bass():
    nc = bacc.Bacc(
        "TRN2", target_bir_lowering=False, debug=False, num_devices=N_CORES
    )
    xT = nc.dram_tensor("xT", [DIM, N], F32R, kind="ExternalInput").ap()
    wq = nc.dram_tensor("wq", [DIM, HDC], F32R, kind="ExternalInput").ap()
    wk = nc.dram_tensor("wk", [DIM, HDC], F32R, kind="ExternalInput").ap()
    wv = nc.dram_tensor("wv", [DIM, HDC], F32R, kind="ExternalInput").ap()
    wo = nc.dram_tensor("wo", [HDC, DIM], F32R, kind="ExternalInput").ap()
    outT = nc.dram_tensor("outT", [DIM, N], F32, kind="ExternalOutput").ap()

    with tile.TileContext(nc) as tc:
        _emit(tc, nc, xT, wq, wk, wv, wo, outT)
    nc.compile()
    return nc


def _emit(tc, nc, xT, wq, wk, wv, wo, outT):
    Exp = mybir.ActivationFunctionType.Exp
    mult = mybir.AluOpType.mult

    with tc.tile_pool(name="qkv", bufs=1) as qkv:
        # ---- persistent tiles -------------------------------------------
        qt = qkv.tile([P, HC, N], F32R)    # Q^T  [2x64 hd, pair, tok]
        kt = qkv.tile([P, HC, N], F32R)    # K^T
        vt = qkv.tile([P, TC, HDC], BF16)  # V    [tok, tok_chunk, hd]
        ot = qkv.tile([P, HC, N], F32R)    # O^T  [2x64 hd, chunk, tok]

        # ---- phase 1: QKV projections -----------------------------------
        with (
            tc.tile_pool(name="w_in", bufs=1) as w_in,
            tc.tile_pool(name="x_in", bufs=2) as x_in,
            tc.tile_pool(name="ps1", bufs=4, space="PSUM") as ps1,
        ):
            wq_sb = w_in.tile([P, CC, HDC], F32R)
            wk_sb = w_in.tile([P, CC, HDC], F32R)
            wv_sb = w_in.tile([P, CC, HDC], F32R)
            nc.sync.dma_start(wq_sb[:], wq.rearrange("(cc ci) m -> ci cc m", ci=P))
            nc.sync.dma_start(wk_sb[:], wk.rearrange("(cc ci) m -> ci cc m", ci=P))
            nc.sync.dma_start(wv_sb[:], wv.rearrange("(cc ci) m -> ci cc m", ci=P))

            XW = 256  # xT staging width (tokens); >=256 keeps fp32r full-rate
            for t in range(N // XW):
                xt_sb = x_in.tile([P, CC, XW], F32R, tag="xt")
                nc.sync.dma_start(
                    xt_sb[:],
                    xT[:, t * XW : (t + 1) * XW].rearrange(
                        "(cc ci) n -> ci cc n", ci=P
                    ),
                )
                # Q^T / K^T: [hd-pair 128, tok 256] accumulated over c
                for pair in range(HC):
                    for dst, w_sb in ((qt, wq_sb), (kt, wk_sb)):
                        ps = ps1.tile([P, XW], F32, tag="ps_qk")
                        for cc in range(CC):
                            nc.tensor.matmul(
                                ps[:],
                                lhsT=w_sb[:, cc, pair * P : (pair + 1) * P],
                                rhs=xt_sb[:, cc, :],
                                start=(cc == 0),
                                stop=(cc == CC - 1),
                            )
                        nc.vector.tensor_copy(
                            dst[:, pair, t * XW : (t + 1) * XW], ps[:]
                        )
                # V token-major: [tok 128, hd 512] accumulated over c
                for s in range(XW // P):
                    tchunk = t * (XW // P) + s
                    ps = ps1.tile([P, HDC], F32, tag="ps_v")
                    for cc in range(CC):
                        nc.tensor.matmul(
                            ps[:],
                            lhsT=xt_sb[:, cc, s * P : (s + 1) * P],
                            rhs=wv_sb[:, cc, :],
                            start=(cc == 0),
                            stop=(cc == CC - 1),
                        )
                    nc.vector.tensor_copy(vt[:, tchunk, :], ps[:])

        # ---- phase 2: attention, 2 quads of 4 heads ---------------------
        with (
            tc.tile_pool(name="pt", bufs=6) as ptp,
            tc.tile_pool(name="rc", bufs=4) as rcp,
            tc.tile_pool(name="aux", bufs=1) as aux,
            tc.tile_pool(name="ps_s", bufs=4, space="PSUM") as ps_s,
            tc.tile_pool(name="ps_o", bufs=2, space="PSUM") as ps_o,
            tc.tile_pool(name="ps_d", bufs=2, space="PSUM") as ps_d,
        ):
            ones_f32 = aux.tile([P, DH], F32)
            nc.vector.memset(ones_f32[:], 1.0)
            ones = aux.tile([P, DH], BF16)
            nc.vector.tensor_copy(ones[:], ones_f32[:])
            for quad in range(2):
                for i in range(TT):
                    po = [
                        ps_o.tile([P, FD], F32, tag="po", name=f"po{k}")
                        for k in range(2)
                    ]
                    # denominators, replicated to 64 rows per head so they
                    # land row-aligned with po[pair]
                    pd = [
                        ps_d.tile([P, FD], F32, tag="pd", name=f"pd{k}")
                        for k in range(2)
                    ]
                    for j in range(TC):
                        pts = []
                        for hq in range(4):       # head within quad
                            pair = quad * 2 + hq // 2
                            hb = (hq % 2) * DH    # 0 or 64: row in pair tile
                            pss = ps_s.tile([P, FD], F32, tag="ps_s")
                            nc.tensor.matmul(
                                pss[:],
                                lhsT=kt[hb : hb + DH, pair, j * P : (j + 1) * P],
                                rhs=qt[hb : hb + DH, pair, i * FD : (i + 1) * FD],
                                start=True,
                                stop=True,
                                tile_position=(hb, 0),
                            )
                            pt = ptp.tile([P, FD], BF16, tag="pt")
                            nc.scalar.activation(pt[:], pss[:], Exp, scale=SCALE)
                            pts.append(pt)
                        for hq in range(4):
                            h_core = quad * 4 + hq  # head index within core
                            hb = (hq % 2) * DH
                            nc.tensor.matmul(
                                po[hq // 2][hb : hb + DH, :],
                                lhsT=vt[:, j, h_core * DH : (h_core + 1) * DH],
                                rhs=pts[hq][:],
                                start=(j == 0),
                                stop=(j == TC - 1),
                                tile_position=(0, hb),
                            )
                            nc.tensor.matmul(
                                pd[hq // 2][hb : hb + DH, :],
                                lhsT=ones[:],
                                rhs=pts[hq][:],
                                start=(j == 0),
                                stop=(j == TC - 1),
                                tile_position=(0, hb),
                            )
                    # normalize O^T by softmax denominators (row-aligned)
                    for pair_i in range(2):
                        chunk = quad * 2 + pair_i
                        rec = rcp.tile([P, FD], F32, tag="rec")
                        nc.vector.reciprocal(rec[:], pd[pair_i][:])
                        nc.vector.tensor_tensor(
                            ot[:, chunk, i * FD : (i + 1) * FD],
                            po[pair_i][:],
                            rec[:],
                            mult,
                        )

        # ---- phase 3: output projection ---------------------------------
        with (
            tc.tile_pool(name="osb", bufs=4) as osb,
            tc.tile_pool(name="wop", bufs=1) as wop,
            tc.tile_pool(name="ps3", bufs=4, space="PSUM") as ps3,
        ):
            wo_sb = wop.tile([P, HC, DIM], F32R)
            nc.sync.dma_start(wo_sb[:], wo.rearrange("(hc hi) o -> hi hc o", hi=P))
            for dc in range(OC):
                for i in range(TT):
                    ps = ps3.tile([P, FD], F32, tag="ps_out")
                    for chunk in range(HC):
                        nc.tensor.matmul(
                            ps[:],
                            lhsT=wo_sb[:, chunk, dc * P : (dc + 1) * P],
                            rhs=ot[:, chunk, i * FD : (i + 1) * FD],
                            start=(chunk == 0),
                            stop=(chunk == HC - 1),
                        )
                    o_sb = osb.tile([P, FD], F32, tag="o_sb")
                    nc.vector.tensor_copy(o_sb[:], ps[:])
                    nc.sync.dma_start(
                        outT[dc * P : (dc + 1) * P, i * FD : (i + 1) * FD], o_sb[:]
                    )


_NC_CACHE = []


def _get_nc():
    if not _NC_CACHE:
        _NC_CACHE.append(build_bass())
    return _NC_CACHE[0]


def shard_inputs(x, Wq, Wk, Wv, Wo):
    in_maps = []
    for c in range(N_CORES):
        b, g = c // 2, c % 2
        sl = slice(g * HDC, (g + 1) * HDC)
        in_maps.append(
            {
                "xT": np.ascontiguousarray(x[b].T),
                "wq": np.ascontiguousarray(Wq[:, sl]),
                "wk": np.ascontiguousarray(Wk[:, sl]),
                "wv": np.ascontiguousarray(Wv[:, sl]),
                "wo": np.ascontiguousarray(Wo[sl, :]),
            }
        )
    return in_maps


def run_sharded(nc, in_maps, **kw):
    return run_bass_kernel_spmd(nc, in_maps, core_ids=list(range(N_CORES)), **kw)


def gather_output(results, bo):
    out = np.empty((B, N, DIM), dtype=np.float32)
    for b in range(B):
        acc = results[2 * b]["outT"] + results[2 * b + 1]["outT"]
        out[b] = acc.T + bo[None, :]
    return out


def kernel(x, Wq, Wk, Wv, Wo, bo):
    x = np.asarray(x, dtype=np.float32)
    nc = _get_nc()
    in_maps = shard_inputs(x, Wq, Wk, Wv, Wo)
    res = run_sharded(nc, in_maps)
    return gather_output(res.results, np.asarray(bo, dtype=np.float32))


if __name__ == "__main__":
    # quick self-run with random inputs
    rng = np.random.default_rng(0)
    x = rng.standard_normal((B, N, DIM), dtype=np.float32)
    s = DIM ** -0.5
    Wq = rng.standard_normal((DIM, DIM), dtype=np.float32) * s
    Wk = rng.standard_normal((DIM, DIM), dtype=np.float32) * s
    Wv = rng.standard_normal((DIM, DIM), dtype=np.float32) * s
    Wo = rng.standard_normal((DIM, DIM), dtype=np.float32) * s
    bo = np.zeros(DIM, dtype=np.float32)
    out = kernel(x, Wq, Wk, Wv, Wo, bo)
    print("out", out.shape, out.dtype, float(np.abs(out).mean()))
